# revision 8
# baseline (speedup 1.0000x reference)
"""Channelwise symmetric Hausdorff distance loss on 8 Trainium2 NeuronCores.

Math (per (batch, channel) pair; x, y are [N, D] point sets):
    d2[n, m] = |x_n|^2 + |y_m|^2 - 2 x_n.y_m
    h = max( max_n min_m d(n,m), max_m min_n d(n,m) )
    answer   = mean over the B*C pairs of h.

Sharding: B*C = 24 pairs, 3 per NeuronCore (data parallel), host gathers.

v3 design (vs v1 baseline): the per-n-tile work is spread over three
engines instead of two DVE passes from fp32 PSUM:
  PE : 8 accumulating fp8 DoubleRow matmuls (-2 x.y) + 1 single-row fold
       matmul per bank adding centered y2 -> psum = d2 - x2 - 1024
  ACT: Identity activation with per-partition bias x2c evacuates psum to
       SBUF bf16: colcand = d2 - 2048  (sole PSUM reader)
  DVE: tensor_reduce min over colcand -> rowaccs[:, nt]  (fwd direction)
       tensor_tensor min colacc = min(colacc, colcand)   (bwd direction)
Inputs are host-relayouted partition-major so every DMA moves 2KB+
contiguous per partition line; x2/y2 are centered by -1024 so the bf16
intermediates quantize to +-2.
Host finishes in float64: fwd2 = max(rowaccs)+2048, bwd2 =
max_m(min_p colacc)+2048, h = sqrt(max(fwd2, bwd2, 0)), mean over pairs.
"""

import numpy as np

B, C, N, D = 8, 3, 1024, 1024
N_CORES = 8
PAIRS = B * C              # 24
PP = PAIRS // N_CORES      # 3 pairs per core
NT = N // 128              # 8 n-tiles (output partition dim)
MBS = 512                  # m block size (one PSUM bank of fp32)
MB = N // MBS              # 2 m-blocks
KT = D // 128              # 8 k-tiles (contraction)

_NC_CACHE = None


def _legalize_sync(nc):
    """This toolchain's walrus accepts at most ONE sync-wait per instruction;
    Tile emits several (e.g. the tail drain waits on every engine/DMA sem).
    Hoist all but the last wait of each instruction into standalone
    InstEventSemaphore instructions on the same engine, inserted just before
    it — semantically identical (the engine blocks on each in turn)."""
    import concourse.mybir as mybir

    n_split = 0
    for fn in nc.m.functions:
        for bb in fn.blocks:
            new_il = []
            for ins in bb.instructions:
                si = ins.sync_info
                if si is not None and si.on_wait and len(si.on_wait) > 1:
                    waits = list(si.on_wait)
                    for k, w in enumerate(waits[:-1]):
                        ev = mybir.InstEventSemaphore(
                            name=f"{ins.name}-evw{k}",
                            engine=ins.engine,
                            ins=[],
                            outs=[],
                            sync_info=mybir.SyncInfo(on_wait=[w], on_update=[]),
                        )
                        new_il.append(ev)
                        n_split += 1
                    si.on_wait = [waits[-1]]
                new_il.append(ins)
            bb.instructions[:] = new_il
    return n_split


def _build_nc():
    import concourse.bass as bass
    import concourse.mybir as mybir
    import concourse.tile as tile

    f16 = mybir.dt.float16
    bf16 = mybir.dt.bfloat16
    f32 = mybir.dt.float32
    f8 = mybir.dt.float8e4
    op_min = mybir.AluOpType.min

    nc = bass.Bass("TRN2", target_bir_lowering=True, debug=False)
    # partition-major host layouts: xt[j, p, k, n] = -2 x[j, n, 128k+p]
    xt_d = nc.dram_tensor("xt", [PP, 128, KT, N], f8, kind="ExternalInput").ap()
    yt_d = nc.dram_tensor("yt", [PP, 128, KT, N], f8, kind="ExternalInput").ap()
    y2a_d = nc.dram_tensor("y2a", [1, PP * N], f16, kind="ExternalInput").ap()
    x2c_d = nc.dram_tensor("x2c", [128, PP * NT], f32, kind="ExternalInput").ap()
    row_d = nc.dram_tensor("rowout", [PP, 128, NT], f32, kind="ExternalOutput").ap()
    col_d = nc.dram_tensor("colout", [PP, 128, N], bf16, kind="ExternalOutput").ap()

    with tile.TileContext(nc) as tc:
        with (
            tc.tile_pool(name="const", bufs=1) as const_pool,
            tc.tile_pool(name="xy", bufs=2) as xy_pool,
            tc.tile_pool(name="small", bufs=2) as small_pool,
            tc.tile_pool(name="cc", bufs=3) as cc_pool,
            tc.tile_pool(name="ps", bufs=4, space="PSUM") as ps_pool,
        ):
            ones1 = const_pool.tile([1, 128], f16)
            nc.vector.memset(ones1, 1.0)
            # PE warmup: a few dataless matmuls queue ahead of the real
            # stream so the HAM clock-gate is (nearly) released by the time
            # the first input chunk lands.
            wu = const_pool.tile([128, 2, MBS], f8)
            nc.vector.memset(wu, 1.0)
            wu_ps = ps_pool.tile([128, MB, MBS], f32, tag="ps")
            for i in range(4):
                nc.tensor.matmul(
                    wu_ps[:, 0, :],
                    wu[:, :, 0:128],
                    wu,
                    start=(i == 0),
                    stop=(i == 3),
                    perf_mode=mybir.MatmulPerfMode.DoubleRow,
                )

            # whole-kernel small inputs, issued on the (early-idle) ACT
            # queue so they don't delay the first xt/yt chunks below.
            x2c_sb = const_pool.tile([128, PP * NT], f32)
            nc.scalar.dma_start(out=x2c_sb, in_=x2c_d)
            y2a_sb = const_pool.tile([1, PP * N], f16)
            nc.scalar.dma_start(out=y2a_sb, in_=y2a_d)

            xt_h, xt_r, yt_h, yt_r = [], [], [], []
            for j in range(PP):
                # head chunk (k 0-1) in its own tile so the first matmuls
                # gate on 1/4 of the traffic; xt on Sync, yt on GpSimd so
                # the two head issues go out in parallel.
                xh = xy_pool.tile([128, 2, N], f8, tag="xth")
                xr = xy_pool.tile([128, KT - 2, N], f8, tag="xtr")
                yh = xy_pool.tile([128, 2, N], f8, tag="yth")
                yr = xy_pool.tile([128, KT - 2, N], f8, tag="ytr")
                nc.sync.dma_start(out=xh, in_=xt_d[j, :, 0:2, :])
                nc.gpsimd.dma_start(out=yh, in_=yt_d[j, :, 0:2, :])
                nc.sync.dma_start(out=xr, in_=xt_d[j, :, 2:KT, :])
                nc.gpsimd.dma_start(out=yr, in_=yt_d[j, :, 2:KT, :])
                xt_h.append(xh)
                xt_r.append(xr)
                yt_h.append(yh)
                yt_r.append(yr)

            for j in range(PP):
                rowaccs = small_pool.tile([128, NT], f32, tag="rowaccs")
                colacc = small_pool.tile([128, N], bf16, tag="colacc")

                for nt in range(NT):
                    nsl = slice(nt * 128, (nt + 1) * 128)
                    ps = ps_pool.tile([128, MB, MBS], f32, tag="ps")
                    for ki in range(KT // 2):
                        if ki == 0:
                            xsl = xt_h[j][:, :, nsl]
                            yt_k = yt_h[j]
                        else:
                            xsl = xt_r[j][:, 2 * ki - 2 : 2 * ki, nsl]
                            yt_k = yt_r[j][:, 2 * ki - 2 : 2 * ki, :]
                        for mb in range(MB):
                            nc.tensor.matmul(
                                ps[:, mb, :],
                                xsl,
                                yt_k[:, :, mb * MBS : (mb + 1) * MBS],
                                start=(ki == 0),
                                stop=False,
                                perf_mode=mybir.MatmulPerfMode.DoubleRow,
                            )
                    # fold in centered y2: psum += 1 * y2a[m]
                    for mb in range(MB):
                        nc.tensor.matmul(
                            ps[:, mb, :],
                            ones1,
                            y2a_sb[:, j * N + mb * MBS : j * N + (mb + 1) * MBS],
                            start=False,
                            stop=True,
                        )
                    # ACT evacuates psum + x2c[n] -> bf16 SBUF (= d2 - 2048)
                    colcand = cc_pool.tile([128, N], bf16, tag="colcand")
                    nc.scalar.activation(
                        colcand.rearrange("p (a m) -> p a m", a=MB),
                        ps,
                        mybir.ActivationFunctionType.Identity,
                        bias=x2c_sb[:, j * NT + nt : j * NT + nt + 1],
                        scale=1.0,
                    )
                    # bwd first (it gates the colacc DMA on the last n-tile)
                    if nt == 0:
                        nc.vector.tensor_copy(colacc, colcand)
                    else:
                        nc.vector.tensor_tensor(
                            out=colacc, in0=colcand, in1=colacc, op=op_min
                        )
                    # fwd: rowaccs[:, nt] = min_m colcand
                    nc.vector.tensor_reduce(
                        out=rowaccs[:, nt : nt + 1],
                        in_=colcand,
                        axis=mybir.AxisListType.X,
                        op=op_min,
                    )
                nc.scalar.dma_start(out=col_d[j], in_=colacc)
                nc.scalar.dma_start(out=row_d[j], in_=rowaccs)
    _legalize_sync(nc)
    return nc


def _prep_inputs(x, y):
    import ml_dtypes

    f8np = np.dtype(ml_dtypes.float8_e4m3)
    x32 = np.ascontiguousarray(x, dtype=np.float32).reshape(PAIRS, N, D)
    y32 = np.ascontiguousarray(y, dtype=np.float32).reshape(PAIRS, N, D)

    # xt[q, p, k, n] = -2 x[q, n, 128k+p]; yt[q, p, k, m] = y[q, m, 128k+p]
    xt8 = np.empty((PAIRS, 128, KT, N), f8np)
    yt8 = np.empty((PAIRS, 128, KT, N), f8np)
    for q in range(PAIRS):
        xt8[q] = (
            (x32[q].T * np.float32(-2.0)).reshape(KT, 128, N).transpose(1, 0, 2)
        ).astype(f8np)
        yt8[q] = (y32[q].T.reshape(KT, 128, N).transpose(1, 0, 2)).astype(f8np)

    x2 = np.square(x32.astype(np.float64)).sum(-1)  # [PAIRS, N]
    y2 = np.square(y32.astype(np.float64)).sum(-1)
    # per-core packed small tensors:
    # x2c[core][p, j*NT + t] = x2[q0+j, t*128 + p] - 1024  (fp32)
    # y2a[core][0, j*N + m]  = y2[q0+j, m] - 1024          (fp16)
    x2c_pairs = (
        (x2 - 1024.0).reshape(PAIRS, NT, 128).transpose(0, 2, 1).astype(np.float32)
    )  # [q, p, t]
    x2c = np.ascontiguousarray(
        x2c_pairs.reshape(N_CORES, PP, 128, NT).transpose(0, 2, 1, 3).reshape(
            N_CORES, 128, PP * NT
        )
    )
    y2a = np.ascontiguousarray(
        (y2 - 1024.0).astype(np.float16).reshape(N_CORES, 1, PP * N)
    )
    return xt8, yt8, x2c, y2a


def _run(x, y, trace=False):
    global _NC_CACHE
    from concourse.bass_utils import run_bass_kernel_spmd

    xt8, yt8, x2c, y2a = _prep_inputs(x, y)

    if _NC_CACHE is None:
        _NC_CACHE = _build_nc()
    nc = _NC_CACHE

    in_maps = []
    for i in range(N_CORES):
        q0 = i * PP
        in_maps.append(
            {
                "xt": xt8[q0 : q0 + PP],
                "yt": yt8[q0 : q0 + PP],
                "y2a": y2a[i],
                "x2c": x2c[i],
            }
        )

    res = run_bass_kernel_spmd(nc, in_maps, core_ids=list(range(N_CORES)), trace=trace)

    h2 = np.empty(PAIRS, np.float64)
    for i in range(N_CORES):
        r = res.results[i]
        for j in range(PP):
            q = i * PP + j
            # rowaccs[p, t] = min_m(d2 - 2048) for n = t*128+p
            fwd2 = r["rowout"][j].astype(np.float64).max() + 2048.0
            # colacc[p, m] = min over n-tiles of (d2 - 2048)
            bwd2 = r["colout"][j].astype(np.float64).min(0).max() + 2048.0
            h2[q] = max(fwd2, bwd2, 0.0)

    ans = np.sqrt(h2).mean()
    return np.array(ans, dtype=np.float32), res


def kernel(input, target):
    out, _ = _run(np.asarray(input), np.asarray(target), trace=False)
    return out
